# revision 18
# baseline (speedup 1.0000x reference)
"""Causal self-attention (single head) on 8 TRN2 NeuronCores.

Sharding: data-parallel over batch (4) x query-interleave (2).
Core c handles batch b = c//2 and the 8 query blocks (128 q each)
J_BLOCKS[c%2]; slot t's block g is in {2t, 2t+1}, so covering key
tiles 0..2t+1 (natural global order) is uniform across cores and the
causal mask beyond the static structure is data-driven.

Algorithm per core (all matmuls bf16 with f32 PSUM):
  QT[d2, q]  = Wqk^T-projected queries (Wqk = Wq^T Wk host-fused), so
               scores = QT^T x_k^T needs no on-chip K projection.
  V'[k, eo]  = x Wvo^T (Wvo = Wo Wv host-fused): attended @ V' IS the
               final output projection.
  scoresT[k, q] computed per key tile g with q spanning all slots
               that cover g -> wide matmuls, and softmax weights come
               out already in [k, q] layout: no transposes at all.
  softmax    = exp without max subtraction (scores ~ N(0,1), safe in
               f32), denominator via ones-column matmul, normalization
               folded into the attended PSUM eviction scale.

USE_CC: V' is computed only for this core's own 8 key blocks and
pair-wise AllGathered (cores 2b/2b+1), halving the biggest matmul.
The V projection runs in two e-column halves (lo: 0..511, hi: 512..)
over all 8 blocks at once (8 PSUM banks), each half feeding its own
AllGather, so the first gather is in flight as early as possible.
"""

from contextlib import ExitStack

import numpy as np
import ml_dtypes

USE_CC = True

B, S, D = 4, 2048, 1024
P = 128
ND = D // P  # 8 contraction chunks
NE = D // P  # 8 feature chunks
NSK = S // P  # 16 key tiles
NQB = 8  # query blocks per core
SQH = NQB * P  # 1024 queries per core
J_BLOCKS = (
    [0, 3, 4, 7, 8, 11, 12, 15],
    [1, 2, 5, 6, 9, 10, 13, 14],
)
# per-half allgather row order: [my 4 blocks (rank 0) | peer 4 (rank 1)];
# half 0 holds global key tiles 0..7, half 1 holds 8..15
VG_POS2 = {}
for _h in range(2):
    for _i, _g in enumerate(J_BLOCKS[0][4 * _h : 4 * _h + 4]
                            + J_BLOCKS[1][4 * _h : 4 * _h + 4]):
        VG_POS2[_g] = _i
SCALE = 1.0 / np.sqrt(np.float32(D))  # 1/32
NEG_BIG = -1.0e30
CPW = SQH + NSK  # packed f32 consts width
H = 512  # e-column half width

_NC = None


def _emit(nc, tc, dr, out_d):
    from concourse import mybir

    BF = mybir.dt.bfloat16
    F32 = mybir.dt.float32
    AF = mybir.ActivationFunctionType
    Alu = mybir.AluOpType

    with ExitStack() as ctx:
        const = ctx.enter_context(tc.tile_pool(name="const", bufs=1))
        cpak = const.tile([P, CPW], F32)
        qpos = cpak[:, 0:SQH]
        kposc = cpak[:, SQH : SQH + NSK]
        cbf = const.tile([P, 8], BF)
        ones1 = cbf[:, 0:1]

        # persistent activation storage; V kept as per-tile e-halves
        xq_pool = ctx.enter_context(tc.tile_pool(name="xq", bufs=2 * ND))
        xt_pool = ctx.enter_context(tc.tile_pool(name="xt", bufs=ND))
        qt_pool = ctx.enter_context(tc.tile_pool(name="qt", bufs=NE))
        v_pool = ctx.enter_context(tc.tile_pool(name="v", bufs=NSK))
        XQ = [[None] * ND, [None] * ND]  # [half][d]: q cols 512h..512h+511
        XT, QT = [None] * ND, []
        V = [None] * NSK

        # ---------------- phase A ----------------
        with ExitStack() as actx:
            wv_pool = actx.enter_context(tc.tile_pool(name="wv", bufs=ND))
            wq_pool = actx.enter_context(tc.tile_pool(name="wq", bufs=ND))
            WV = [None] * ND
            WQ = [None] * ND

            # input streams on the 3 DMA rings (sync/scalar/gpsimd),
            # interleaved by need order (V pass-A d-step d needs xq[d]
            # + wv[d]).  The scalar (Activation) engine gets ONLY a
            # short non-blocking prefix of triggers: a blocked trigger
            # in its queue would delay every PSUM eviction behind it.
            # gpsimd owns the collective pipeline (vin writes, gathers,
            # readbacks) so no other ring ever blocks on a gather.
            RINGS = (nc.sync, nc.scalar, nc.gpsimd)

            def load_xq(d, ring, h):
                t = xq_pool.tile([P, H], BF, name="xq")
                RINGS[ring].dma_start(t[:], dr["xq"][d][:, h * H : (h + 1) * H])
                XQ[h][d] = t

            def load_wv(d, ring):
                t = wv_pool.tile([P, D], BF, name="wv")
                RINGS[ring].dma_start(t[:], dr["wvT"][d])
                WV[d] = t

            # scalar's queue head holds the exp ACT_TABLE_LOAD, so its
            # first transfer lands ~1.3us later than the other rings:
            # keep the d0/d1 operands off it.  Pass A (key blocks 0..3)
            # reads only the lo half of xq, so xq-hi streams later.
            # d0 operands in need-sized chunks: the first matmul only
            # touches xq0-lo[:, 0:128] and wv0[:, 0:512]
            t = xq_pool.tile([P, H], BF, name="xq")
            nc.sync.dma_start(t[:, 0:P], dr["xq"][0][:, 0:P])
            nc.sync.dma_start(t[:, P:H], dr["xq"][0][:, P:H])
            XQ[0][0] = t
            t = wv_pool.tile([P, D], BF, name="wv")
            nc.gpsimd.dma_start(t[:, 0:512], dr["wvT"][0][:, 0:512])
            nc.gpsimd.dma_start(t[:, 512:D], dr["wvT"][0][:, 512:D])
            WV[0] = t
            for key, d, ring in (
                ("wv", 1, 2), ("xq", 1, 1),
                ("wv", 2, 0), ("xq", 2, 2), ("xq", 3, 0), ("wv", 3, 1),
                ("xq", 4, 1), ("wv", 4, 2), ("wv", 5, 0), ("xq", 5, 2),
                ("xq", 6, 0), ("wv", 6, 1), ("xq", 7, 1), ("wv", 7, 2),
            ):
                (load_wv(d, ring) if key == "wv" else load_xq(d, ring, 0))
            for d in range(ND):
                load_xq(d, (d % 2), 1)
            nc.gpsimd.dma_start(cpak[:], dr["cpak"])
            nc.gpsimd.dma_start(cbf[:], dr["cbf"])
            for d in range(ND):
                WQ[d] = wq_pool.tile([P, D], BF, name="wq")
                nc.sync.dma_start(WQ[d][:], dr["wq2"][d])
            for d in range(4):
                t = xt_pool.tile([P, S], BF, name="xt")
                nc.sync.dma_start(t[:], dr["xT"][d])
                XT[d] = t
            # xT[4..7] go on gpsimd AFTER the gather doorbells (below)

            aps = actx.enter_context(tc.tile_pool(name="aps", bufs=8, space="PSUM"))

            if USE_CC:
                # ---- V' for my 8 blocks (from xq), d-outer in 2 passes
                # of 4 full-width tiles.  Each pass's 4 blocks, paired
                # with the peer's, are exactly global key tiles 0..7
                # (pass 0) / 8..15 (pass 1), so each pass feeds its own
                # AllGather while later compute proceeds. ----
                dram = actx.enter_context(
                    tc.tile_pool(name="dram", bufs=1, space="DRAM")
                )
                vins = [dram.tile([4 * P, D], BF, name=f"vin{h}") for h in range(2)]
                vgs = [
                    dram.tile([8 * P, D], BF, name=f"vg{h}") for h in range(2)
                ]
                vsb = actx.enter_context(tc.tile_pool(name="vsb", bufs=NQB))
                for half in range(2):
                    pss = [
                        [aps.tile([P, 512], F32, tag="ps", name="ps") for _ in range(2)]
                        for _ in range(4)
                    ]
                    for d in range(ND):
                        for si in range(4):
                            s = half * 4 + si
                            for nt in range(2):
                                nc.tensor.matmul(
                                    pss[si][nt][:],
                                    XQ[half][d][:, si * P : (si + 1) * P],
                                    WV[d][:, nt * 512 : (nt + 1) * 512],
                                    start=(d == 0),
                                    stop=(d == ND - 1),
                                )
                    for si in range(4):
                        vt = vsb.tile([P, D], BF, name="vsb")
                        for nt in range(2):
                            nc.scalar.activation(
                                vt[:, nt * 512 : (nt + 1) * 512],
                                pss[si][nt][:],
                                AF.Copy,
                            )
                        nc.gpsimd.dma_start(vins[half][si * P : (si + 1) * P, :], vt[:])
                    nc.gpsimd.collective_compute(
                        "AllGather",
                        mybir.AluOpType.bypass,
                        replica_groups=[[0, 1], [2, 3], [4, 5], [6, 7]],
                        ins=[vins[half].opt()],
                        outs=[vgs[half].opt()],
                    )
                # xT[4..7]: fire right after the doorbells, land well
                # before the score phase needs them
                for d in range(4, ND):
                    t = xt_pool.tile([P, S], BF, name="xt")
                    nc.gpsimd.dma_start(t[:], dr["xT"][d])
                    XT[d] = t
                # readback in ascending global-block order (attend order);
                # gather h rows = [my 4 blocks | peer's 4] of half h
                for g in range(NSK):
                    h, i = g // 8, VG_POS2[g]
                    vt = v_pool.tile([P, D], BF, name="v")
                    nc.gpsimd.dma_start(vt[:], vgs[h][i * P : (i + 1) * P, :])
                    V[g] = vt
            else:
                for d in range(4, ND):
                    t = xt_pool.tile([P, S], BF, name="xt")
                    nc.sync.dma_start(t[:], dr["xT"][d])
                    XT[d] = t
                # ---- V' for all 16 tiles from xT, s-outer ----
                for s in range(NSK):
                    vt = v_pool.tile([P, D], BF, name="v")
                    pss = [aps.tile([P, 512], F32, tag="ps", name="ps") for _ in range(2)]
                    for d in range(ND):
                        for nt in range(2):
                            nc.tensor.matmul(
                                pss[nt][:],
                                XT[d][:, s * P : (s + 1) * P],
                                WV[d][:, nt * 512 : (nt + 1) * 512],
                                start=(d == 0),
                                stop=(d == ND - 1),
                            )
                    for nt in range(2):
                        nc.scalar.activation(
                            vt[:, nt * 512 : (nt + 1) * 512], pss[nt][:], AF.Copy
                        )
                    V[s] = vt

            # ---- QT, e-outer; one LDWEIGHTS per (e, d) for both halves ----
            for e in range(NE):
                qts = qt_pool.tile([P, SQH], BF, name="qt")
                qp = [aps.tile([P, 512], F32, tag="ps", name="ps") for _ in range(2)]
                for d in range(ND):
                    for hq in range(2):
                        nc.tensor.matmul(
                            qp[hq][:],
                            WQ[d][:, e * P : (e + 1) * P],
                            XQ[hq][d][:],
                            start=(d == 0),
                            stop=(d == ND - 1),
                        )
                for hq in range(2):
                    nc.scalar.activation(
                        qts[:, hq * 512 : (hq + 1) * 512], qp[hq][:], AF.Copy
                    )
                QT.append(qts)

        # ---------------- phase B: scores + attend ----------------
        with ExitStack() as bctx:
            maskp = bctx.enter_context(tc.tile_pool(name="mask", bufs=2))
            smp = bctx.enter_context(tc.tile_pool(name="sm", bufs=2))
            expA = bctx.enter_context(tc.tile_pool(name="expA", bufs=8))
            expB = bctx.enter_context(tc.tile_pool(name="expB", bufs=8))
            statp = bctx.enter_context(tc.tile_pool(name="stat", bufs=2))
            outp = bctx.enter_context(tc.tile_pool(name="out", bufs=2))
            ps_s = bctx.enter_context(tc.tile_pool(name="ps_s", bufs=4, space="PSUM"))
            ps_a = bctx.enter_context(tc.tile_pool(name="ps_a", bufs=3, space="PSUM"))
            denp = bctx.enter_context(tc.tile_pool(name="den", bufs=1, space="PSUM"))
            EXP = [None] * NSK

            def emit_scores(g):
                t0 = g // 2
                span = (NQB - t0) * P
                qoff = t0 * P
                # additive causal mask for the slot-t0 q-slice of this tile
                mk = maskp.tile([P, P], F32, tag="mk", name="mk")
                nc.vector.tensor_scalar(
                    mk[:],
                    qpos[:, qoff : qoff + P],
                    kposc[:, g : g + 1],
                    NEG_BIG,
                    op0=Alu.is_lt,
                    op1=Alu.mult,
                )
                pool = expA if span > 512 else expB
                ex = pool.tile([P, max(span, 512)], BF, name="ex")
                EXP[g] = ex
                pieces = [
                    (ps_s.tile([P, 512], F32, tag="ps", name="ps"), off,
                     min(512, span - off))
                    for off in range(0, span, 512)
                ]
                for e in range(NE):
                    for ps, off, w in pieces:
                        nc.tensor.matmul(
                            ps[:, 0:w],
                            XT[e][:, g * P : (g + 1) * P],
                            QT[e][:, qoff + off : qoff + off + w],
                            start=(e == 0),
                            stop=(e == NE - 1),
                        )
                for ps, off, w in pieces:
                    if off == 0:
                        sm = smp.tile([P, P], F32, tag="sm", name="sm")
                        nc.vector.tensor_tensor(sm[:], ps[:, 0:P], mk[:], op=Alu.add)
                        nc.scalar.activation(
                            ex[:, 0:P], sm[:], AF.Exp, scale=float(SCALE)
                        )
                        if w > P:
                            nc.scalar.activation(
                                ex[:, P:w], ps[:, P:w], AF.Exp, scale=float(SCALE)
                            )
                    else:
                        nc.scalar.activation(
                            ex[:, off : off + w],
                            ps[:, 0:w],
                            AF.Exp,
                            scale=float(SCALE),
                        )

            def emit_attend(t):
                nk = 2 * t + 2
                dps = denp.tile([P, 1], F32, tag="dp", name="dp")
                pas = [ps_a.tile([P, 512], F32, tag="pa", name="pa") for _ in range(2)]
                for g2 in range(nk):
                    lt = EXP[g2][:, (t - g2 // 2) * P : (t - g2 // 2 + 1) * P]
                    nc.tensor.matmul(
                        dps[:], lt, ones1, start=(g2 == 0), stop=(g2 == nk - 1)
                    )
                    for h in range(2):
                        nc.tensor.matmul(
                            pas[h][:],
                            lt,
                            V[g2][:, h * 512 : (h + 1) * 512],
                            start=(g2 == 0),
                            stop=(g2 == nk - 1),
                        )
                rinv = statp.tile([P, 1], F32, tag="ri", name="ri")
                nc.vector.reciprocal(rinv[:], dps[:])
                ob = outp.tile([P, D], BF, tag="ob", name="ob")
                nc.scalar.activation(
                    ob[:, 0:512], pas[0][:], AF.Copy, scale=rinv[:]
                )
                nc.sync.dma_start(out_d[t][:, 0:512], ob[:, 0:512])
                nc.vector.tensor_scalar_mul(ob[:, 512:D], pas[1][:], rinv[:])
                nc.sync.dma_start(out_d[t][:, 512:D], ob[:, 512:D])

            # lagged interleave: A(t) emitted after S(2t+7), so the PE
            # never reaches an attend before its gathered V has landed
            for g in range(NSK):
                emit_scores(g)
                if g >= 7 and g % 2 == 1:
                    emit_attend((g - 7) // 2)
            for t in range(NQB - 3, NQB):
                emit_attend(t)


def build_nc():
    """Build + compile the SPMD Bass program (cached)."""
    global _NC
    if _NC is not None:
        return _NC
    from concourse import bacc, mybir
    import concourse.tile as tile

    BF = mybir.dt.bfloat16
    F32 = mybir.dt.float32

    nc = bacc.Bacc(
        "TRN2",
        target_bir_lowering=False,
        debug=False,
        enable_asserts=False,
        num_devices=8,
    )
    dr = {}

    def din(name, shape, dt):
        dr[name] = nc.dram_tensor(name, shape, dt, kind="ExternalInput").ap()

    din("xT", (ND, P, S), BF)
    din("xq", (ND, P, SQH), BF)
    din("wq2", (ND, P, D), BF)
    din("wvT", (ND, P, D), BF)
    din("cpak", (P, CPW), F32)
    din("cbf", (P, 8), BF)
    out_d = nc.dram_tensor("out_c", (NQB, P, D), BF, kind="ExternalOutput").ap()

    with tile.TileContext(nc) as tc:
        _emit(nc, tc, dr, out_d)
    nc.compile()
    _NC = nc
    return nc


def make_in_maps(x, Wq, bq, Wk, bk, Wv, bv, Wo, bo):
    """Host-side sharding: per-core input dicts (bf16 compute operands)."""
    bf16 = ml_dtypes.bfloat16
    f32 = np.float32

    # host-fused weights (f32 GEMMs, exact up to fp32):
    #   scores = (x Wq^T)(x Wk^T)^T = x (Wq^T Wk) x^T       -> Wqk
    #   out    = softmax(..) (x Wv^T) Wo^T = softmax(..) x (Wo Wv)^T
    # Requires bq = bk = 0 (guaranteed by the problem spec).
    Wqk = Wq.T.astype(np.float32) @ Wk.astype(np.float32)  # [d1, d2]
    Wvo = Wo.astype(np.float32) @ Wv.astype(np.float32)  # [eo, d]
    wq2 = np.ascontiguousarray(Wqk.reshape(ND, P, D)).astype(bf16)
    wvT = np.ascontiguousarray(Wvo.T.reshape(ND, P, D)).astype(bf16)
    kposc = (np.arange(NSK, dtype=f32) * P)[None, :] + np.arange(P, dtype=f32)[
        :, None
    ]  # [P, NSK]: kposc[p, g] = g*128 + p
    cbf = np.ones((P, 8), dtype=bf16)

    in_maps = []
    for c in range(8):
        b, j = c // 2, c % 2
        blocks = J_BLOCKS[j]
        xTb = np.ascontiguousarray(x[b].T)  # [D, S] natural key order
        qcols = np.concatenate([np.r_[P * g : P * (g + 1)] for g in blocks])
        xqb = np.ascontiguousarray(xTb[:, qcols])  # [D, SQH]
        qpos = np.broadcast_to(qcols.astype(f32), (P, SQH))
        cpak = np.concatenate([qpos, kposc], axis=1)
        in_maps.append(
            {
                "xT": xTb.reshape(ND, P, S).astype(bf16),
                "xq": xqb.reshape(ND, P, SQH).astype(bf16),
                "wq2": wq2,
                "wvT": wvT,
                "cpak": np.ascontiguousarray(cpak.astype(f32)),
                "cbf": cbf,
            }
        )
    return in_maps


def assemble_out(results, bvo):
    out = np.empty((B, S, D), dtype=np.float32)
    for c in range(8):
        b, j = c // 2, c % 2
        blocks = J_BLOCKS[j]
        oc = results[c]["out_c"]  # (8, 128, 1024) bf16
        for t, g in enumerate(blocks):
            out[b, P * g : P * (g + 1), :] = oc[t].astype(np.float32)
    if bvo is not None:
        out += bvo[None, None, :]
    return out


def kernel(x, Wq, bq, Wk, bk, Wv, bv, Wo, bo):
    from concourse.bass_utils import run_bass_kernel_spmd

    nc = build_nc()
    in_maps = make_in_maps(x, Wq, bq, Wk, bk, Wv, bv, Wo, bo)
    res = run_bass_kernel_spmd(nc, in_maps, core_ids=list(range(8)))
    bvo = Wo.astype(np.float32) @ bv.astype(np.float32) + bo.astype(np.float32)
    return assemble_out(res.results, bvo)


# revision 19
# speedup vs baseline: 1.0031x; 1.0031x over previous
"""Causal self-attention (single head) on 8 TRN2 NeuronCores.

Sharding: data-parallel over batch (4) x query-interleave (2).
Core c handles batch b = c//2 and the 8 query blocks (128 q each)
J_BLOCKS[c%2]; slot t's block g is in {2t, 2t+1}, so covering key
tiles 0..2t+1 (natural global order) is uniform across cores and the
causal mask beyond the static structure is data-driven.

Algorithm per core (all matmuls bf16 with f32 PSUM):
  QT[d2, q]  = Wqk^T-projected queries (Wqk = Wq^T Wk host-fused), so
               scores = QT^T x_k^T needs no on-chip K projection.
  V'[k, eo]  = x Wvo^T (Wvo = Wo Wv host-fused): attended @ V' IS the
               final output projection.
  scoresT[k, q] computed per key tile g with q spanning all slots
               that cover g -> wide matmuls, and softmax weights come
               out already in [k, q] layout: no transposes at all.
  softmax    = exp without max subtraction (scores ~ N(0,1), safe in
               f32), denominator via ones-column matmul, normalization
               folded into the attended PSUM eviction scale.

USE_CC: V' is computed only for this core's own 8 key blocks and
pair-wise AllGathered (cores 2b/2b+1), halving the biggest matmul.
The V projection runs in two e-column halves (lo: 0..511, hi: 512..)
over all 8 blocks at once (8 PSUM banks), each half feeding its own
AllGather, so the first gather is in flight as early as possible.
"""

from contextlib import ExitStack

import numpy as np
import ml_dtypes

USE_CC = True

B, S, D = 4, 2048, 1024
P = 128
ND = D // P  # 8 contraction chunks
NE = D // P  # 8 feature chunks
NSK = S // P  # 16 key tiles
NQB = 8  # query blocks per core
SQH = NQB * P  # 1024 queries per core
J_BLOCKS = (
    [0, 3, 4, 7, 8, 11, 12, 15],
    [1, 2, 5, 6, 9, 10, 13, 14],
)
# per-half allgather row order: [my 4 blocks (rank 0) | peer 4 (rank 1)];
# half 0 holds global key tiles 0..7, half 1 holds 8..15
VG_POS2 = {}
for _h in range(2):
    for _i, _g in enumerate(J_BLOCKS[0][4 * _h : 4 * _h + 4]
                            + J_BLOCKS[1][4 * _h : 4 * _h + 4]):
        VG_POS2[_g] = _i
SCALE = 1.0 / np.sqrt(np.float32(D))  # 1/32
NEG_BIG = -1.0e30
CPW = SQH + NSK  # packed f32 consts width
H = 512  # e-column half width

_NC = None


def _emit(nc, tc, dr, out_d):
    from concourse import mybir

    BF = mybir.dt.bfloat16
    F32 = mybir.dt.float32
    AF = mybir.ActivationFunctionType
    Alu = mybir.AluOpType

    with ExitStack() as ctx:
        const = ctx.enter_context(tc.tile_pool(name="const", bufs=1))
        cpak = const.tile([P, CPW], F32)
        qpos = cpak[:, 0:SQH]
        kposc = cpak[:, SQH : SQH + NSK]
        cbf = const.tile([P, 8], BF)
        ones1 = cbf[:, 0:1]

        # persistent activation storage; V kept as per-tile e-halves
        xq_pool = ctx.enter_context(tc.tile_pool(name="xq", bufs=2 * ND))
        xt_pool = ctx.enter_context(tc.tile_pool(name="xt", bufs=ND))
        qt_pool = ctx.enter_context(tc.tile_pool(name="qt", bufs=NE))
        v_pool = ctx.enter_context(tc.tile_pool(name="v", bufs=NSK))
        XQ = [[None] * ND, [None] * ND]  # [half][d]: q cols 512h..512h+511
        XT, QT = [None] * ND, []
        V = [None] * NSK

        # ---------------- phase A ----------------
        with ExitStack() as actx:
            wv_pool = actx.enter_context(tc.tile_pool(name="wv", bufs=ND))
            wq_pool = actx.enter_context(tc.tile_pool(name="wq", bufs=ND))
            WV = [None] * ND
            WQ = [None] * ND

            # input streams on the 3 DMA rings (sync/scalar/gpsimd),
            # interleaved by need order (V pass-A d-step d needs xq[d]
            # + wv[d]).  The scalar (Activation) engine gets ONLY a
            # short non-blocking prefix of triggers: a blocked trigger
            # in its queue would delay every PSUM eviction behind it.
            # gpsimd owns the collective pipeline (vin writes, gathers,
            # readbacks) so no other ring ever blocks on a gather.
            RINGS = (nc.sync, nc.scalar, nc.gpsimd)

            def load_xq(d, ring, h):
                t = xq_pool.tile([P, H], BF, name="xq")
                RINGS[ring].dma_start(t[:], dr["xq"][d][:, h * H : (h + 1) * H])
                XQ[h][d] = t

            def load_wv(d, ring):
                t = wv_pool.tile([P, D], BF, name="wv")
                RINGS[ring].dma_start(t[:], dr["wvT"][d])
                WV[d] = t

            # scalar's queue head holds the exp ACT_TABLE_LOAD, so its
            # first transfer lands ~1.3us later than the other rings:
            # keep the d0/d1 operands off it.  Pass A (key blocks 0..3)
            # reads only the lo half of xq, so xq-hi streams later.
            for key, d, ring in (
                ("xq", 0, 0), ("wv", 0, 2), ("wv", 1, 2), ("xq", 1, 1),
                ("wv", 2, 0), ("xq", 2, 2), ("xq", 3, 0), ("wv", 3, 1),
                ("xq", 4, 1), ("wv", 4, 2), ("wv", 5, 0), ("xq", 5, 2),
                ("xq", 6, 0), ("wv", 6, 1), ("xq", 7, 1), ("wv", 7, 2),
            ):
                (load_wv(d, ring) if key == "wv" else load_xq(d, ring, 0))
            for d in range(ND):
                load_xq(d, (d % 2), 1)
            nc.gpsimd.dma_start(cpak[:], dr["cpak"])
            nc.gpsimd.dma_start(cbf[:], dr["cbf"])
            for d in range(ND):
                WQ[d] = wq_pool.tile([P, D], BF, name="wq")
                nc.sync.dma_start(WQ[d][:], dr["wq2"][d])
            for d in range(4):
                t = xt_pool.tile([P, S], BF, name="xt")
                nc.sync.dma_start(t[:], dr["xT"][d])
                XT[d] = t
            # xT[4..7] go on gpsimd AFTER the gather doorbells (below)

            aps = actx.enter_context(tc.tile_pool(name="aps", bufs=8, space="PSUM"))

            if USE_CC:
                # ---- V' for my 8 blocks (from xq), d-outer in 2 passes
                # of 4 full-width tiles.  Each pass's 4 blocks, paired
                # with the peer's, are exactly global key tiles 0..7
                # (pass 0) / 8..15 (pass 1), so each pass feeds its own
                # AllGather while later compute proceeds. ----
                dram = actx.enter_context(
                    tc.tile_pool(name="dram", bufs=1, space="DRAM")
                )
                vins = [dram.tile([4 * P, D], BF, name=f"vin{h}") for h in range(2)]
                vgs = [
                    dram.tile([8 * P, D], BF, name=f"vg{h}") for h in range(2)
                ]
                vsb = actx.enter_context(tc.tile_pool(name="vsb", bufs=NQB))
                for half in range(2):
                    pss = [
                        [aps.tile([P, 512], F32, tag="ps", name="ps") for _ in range(2)]
                        for _ in range(4)
                    ]
                    for d in range(ND):
                        for si in range(4):
                            s = half * 4 + si
                            for nt in range(2):
                                nc.tensor.matmul(
                                    pss[si][nt][:],
                                    XQ[half][d][:, si * P : (si + 1) * P],
                                    WV[d][:, nt * 512 : (nt + 1) * 512],
                                    start=(d == 0),
                                    stop=(d == ND - 1),
                                )
                    for si in range(4):
                        vt = vsb.tile([P, D], BF, name="vsb")
                        for nt in range(2):
                            nc.scalar.activation(
                                vt[:, nt * 512 : (nt + 1) * 512],
                                pss[si][nt][:],
                                AF.Copy,
                            )
                        nc.gpsimd.dma_start(vins[half][si * P : (si + 1) * P, :], vt[:])
                    nc.gpsimd.collective_compute(
                        "AllGather",
                        mybir.AluOpType.bypass,
                        replica_groups=[[0, 1], [2, 3], [4, 5], [6, 7]],
                        ins=[vins[half].opt()],
                        outs=[vgs[half].opt()],
                    )
                # xT[4..7]: fire right after the doorbells, land well
                # before the score phase needs them
                for d in range(4, ND):
                    t = xt_pool.tile([P, S], BF, name="xt")
                    nc.gpsimd.dma_start(t[:], dr["xT"][d])
                    XT[d] = t
                # readback in ascending global-block order (attend order);
                # gather h rows = [my 4 blocks | peer's 4] of half h
                for g in range(NSK):
                    h, i = g // 8, VG_POS2[g]
                    vt = v_pool.tile([P, D], BF, name="v")
                    nc.gpsimd.dma_start(vt[:], vgs[h][i * P : (i + 1) * P, :])
                    V[g] = vt
            else:
                for d in range(4, ND):
                    t = xt_pool.tile([P, S], BF, name="xt")
                    nc.sync.dma_start(t[:], dr["xT"][d])
                    XT[d] = t
                # ---- V' for all 16 tiles from xT, s-outer ----
                for s in range(NSK):
                    vt = v_pool.tile([P, D], BF, name="v")
                    pss = [aps.tile([P, 512], F32, tag="ps", name="ps") for _ in range(2)]
                    for d in range(ND):
                        for nt in range(2):
                            nc.tensor.matmul(
                                pss[nt][:],
                                XT[d][:, s * P : (s + 1) * P],
                                WV[d][:, nt * 512 : (nt + 1) * 512],
                                start=(d == 0),
                                stop=(d == ND - 1),
                            )
                    for nt in range(2):
                        nc.scalar.activation(
                            vt[:, nt * 512 : (nt + 1) * 512], pss[nt][:], AF.Copy
                        )
                    V[s] = vt

            # ---- QT, e-outer; one LDWEIGHTS per (e, d) for both halves ----
            for e in range(NE):
                qts = qt_pool.tile([P, SQH], BF, name="qt")
                qp = [aps.tile([P, 512], F32, tag="ps", name="ps") for _ in range(2)]
                for d in range(ND):
                    for hq in range(2):
                        nc.tensor.matmul(
                            qp[hq][:],
                            WQ[d][:, e * P : (e + 1) * P],
                            XQ[hq][d][:],
                            start=(d == 0),
                            stop=(d == ND - 1),
                        )
                for hq in range(2):
                    nc.scalar.activation(
                        qts[:, hq * 512 : (hq + 1) * 512], qp[hq][:], AF.Copy
                    )
                QT.append(qts)

        # ---------------- phase B: scores + attend ----------------
        with ExitStack() as bctx:
            maskp = bctx.enter_context(tc.tile_pool(name="mask", bufs=2))
            smp = bctx.enter_context(tc.tile_pool(name="sm", bufs=2))
            expA = bctx.enter_context(tc.tile_pool(name="expA", bufs=8))
            expB = bctx.enter_context(tc.tile_pool(name="expB", bufs=8))
            statp = bctx.enter_context(tc.tile_pool(name="stat", bufs=2))
            outp = bctx.enter_context(tc.tile_pool(name="out", bufs=2))
            ps_s = bctx.enter_context(tc.tile_pool(name="ps_s", bufs=4, space="PSUM"))
            ps_a = bctx.enter_context(tc.tile_pool(name="ps_a", bufs=3, space="PSUM"))
            denp = bctx.enter_context(tc.tile_pool(name="den", bufs=1, space="PSUM"))
            EXP = [None] * NSK

            def emit_scores(g):
                t0 = g // 2
                span = (NQB - t0) * P
                qoff = t0 * P
                # additive causal mask for the slot-t0 q-slice of this tile
                mk = maskp.tile([P, P], F32, tag="mk", name="mk")
                nc.vector.tensor_scalar(
                    mk[:],
                    qpos[:, qoff : qoff + P],
                    kposc[:, g : g + 1],
                    NEG_BIG,
                    op0=Alu.is_lt,
                    op1=Alu.mult,
                )
                pool = expA if span > 512 else expB
                ex = pool.tile([P, max(span, 512)], BF, name="ex")
                EXP[g] = ex
                pieces = [
                    (ps_s.tile([P, 512], F32, tag="ps", name="ps"), off,
                     min(512, span - off))
                    for off in range(0, span, 512)
                ]
                for e in range(NE):
                    for ps, off, w in pieces:
                        nc.tensor.matmul(
                            ps[:, 0:w],
                            XT[e][:, g * P : (g + 1) * P],
                            QT[e][:, qoff + off : qoff + off + w],
                            start=(e == 0),
                            stop=(e == NE - 1),
                        )
                for ps, off, w in pieces:
                    if off == 0:
                        sm = smp.tile([P, P], F32, tag="sm", name="sm")
                        nc.vector.tensor_tensor(sm[:], ps[:, 0:P], mk[:], op=Alu.add)
                        nc.scalar.activation(
                            ex[:, 0:P], sm[:], AF.Exp, scale=float(SCALE)
                        )
                        if w > P:
                            nc.scalar.activation(
                                ex[:, P:w], ps[:, P:w], AF.Exp, scale=float(SCALE)
                            )
                    else:
                        nc.scalar.activation(
                            ex[:, off : off + w],
                            ps[:, 0:w],
                            AF.Exp,
                            scale=float(SCALE),
                        )

            def emit_attend(t):
                nk = 2 * t + 2
                dps = denp.tile([P, 1], F32, tag="dp", name="dp")
                pas = [ps_a.tile([P, 512], F32, tag="pa", name="pa") for _ in range(2)]
                for g2 in range(nk):
                    lt = EXP[g2][:, (t - g2 // 2) * P : (t - g2 // 2 + 1) * P]
                    nc.tensor.matmul(
                        dps[:], lt, ones1, start=(g2 == 0), stop=(g2 == nk - 1)
                    )
                    for h in range(2):
                        nc.tensor.matmul(
                            pas[h][:],
                            lt,
                            V[g2][:, h * 512 : (h + 1) * 512],
                            start=(g2 == 0),
                            stop=(g2 == nk - 1),
                        )
                rinv = statp.tile([P, 1], F32, tag="ri", name="ri")
                nc.vector.reciprocal(rinv[:], dps[:])
                ob = outp.tile([P, D], BF, tag="ob", name="ob")
                nc.scalar.activation(
                    ob[:, 0:512], pas[0][:], AF.Copy, scale=rinv[:]
                )
                nc.sync.dma_start(out_d[t][:, 0:512], ob[:, 0:512])
                nc.vector.tensor_scalar_mul(ob[:, 512:D], pas[1][:], rinv[:])
                nc.sync.dma_start(out_d[t][:, 512:D], ob[:, 512:D])

            # lagged interleave: A(t) emitted after S(2t+6), so the PE
            # never reaches an attend before its gathered V has landed
            for g in range(NSK):
                emit_scores(g)
                if g >= 6 and g % 2 == 0:
                    emit_attend((g - 6) // 2)
            for t in range(NQB - 3, NQB):
                emit_attend(t)


def build_nc():
    """Build + compile the SPMD Bass program (cached)."""
    global _NC
    if _NC is not None:
        return _NC
    from concourse import bacc, mybir
    import concourse.tile as tile

    BF = mybir.dt.bfloat16
    F32 = mybir.dt.float32

    nc = bacc.Bacc(
        "TRN2",
        target_bir_lowering=False,
        debug=False,
        enable_asserts=False,
        num_devices=8,
    )
    dr = {}

    def din(name, shape, dt):
        dr[name] = nc.dram_tensor(name, shape, dt, kind="ExternalInput").ap()

    din("xT", (ND, P, S), BF)
    din("xq", (ND, P, SQH), BF)
    din("wq2", (ND, P, D), BF)
    din("wvT", (ND, P, D), BF)
    din("cpak", (P, CPW), F32)
    din("cbf", (P, 8), BF)
    out_d = nc.dram_tensor("out_c", (NQB, P, D), BF, kind="ExternalOutput").ap()

    with tile.TileContext(nc) as tc:
        _emit(nc, tc, dr, out_d)
    nc.compile()
    _NC = nc
    return nc


def make_in_maps(x, Wq, bq, Wk, bk, Wv, bv, Wo, bo):
    """Host-side sharding: per-core input dicts (bf16 compute operands)."""
    bf16 = ml_dtypes.bfloat16
    f32 = np.float32

    # host-fused weights (f32 GEMMs, exact up to fp32):
    #   scores = (x Wq^T)(x Wk^T)^T = x (Wq^T Wk) x^T       -> Wqk
    #   out    = softmax(..) (x Wv^T) Wo^T = softmax(..) x (Wo Wv)^T
    # Requires bq = bk = 0 (guaranteed by the problem spec).
    Wqk = Wq.T.astype(np.float32) @ Wk.astype(np.float32)  # [d1, d2]
    Wvo = Wo.astype(np.float32) @ Wv.astype(np.float32)  # [eo, d]
    wq2 = np.ascontiguousarray(Wqk.reshape(ND, P, D)).astype(bf16)
    wvT = np.ascontiguousarray(Wvo.T.reshape(ND, P, D)).astype(bf16)
    kposc = (np.arange(NSK, dtype=f32) * P)[None, :] + np.arange(P, dtype=f32)[
        :, None
    ]  # [P, NSK]: kposc[p, g] = g*128 + p
    cbf = np.ones((P, 8), dtype=bf16)

    in_maps = []
    for c in range(8):
        b, j = c // 2, c % 2
        blocks = J_BLOCKS[j]
        xTb = np.ascontiguousarray(x[b].T)  # [D, S] natural key order
        qcols = np.concatenate([np.r_[P * g : P * (g + 1)] for g in blocks])
        xqb = np.ascontiguousarray(xTb[:, qcols])  # [D, SQH]
        qpos = np.broadcast_to(qcols.astype(f32), (P, SQH))
        cpak = np.concatenate([qpos, kposc], axis=1)
        in_maps.append(
            {
                "xT": xTb.reshape(ND, P, S).astype(bf16),
                "xq": xqb.reshape(ND, P, SQH).astype(bf16),
                "wq2": wq2,
                "wvT": wvT,
                "cpak": np.ascontiguousarray(cpak.astype(f32)),
                "cbf": cbf,
            }
        )
    return in_maps


def assemble_out(results, bvo):
    out = np.empty((B, S, D), dtype=np.float32)
    for c in range(8):
        b, j = c // 2, c % 2
        blocks = J_BLOCKS[j]
        oc = results[c]["out_c"]  # (8, 128, 1024) bf16
        for t, g in enumerate(blocks):
            out[b, P * g : P * (g + 1), :] = oc[t].astype(np.float32)
    if bvo is not None:
        out += bvo[None, None, :]
    return out


def kernel(x, Wq, bq, Wk, bk, Wv, bv, Wo, bo):
    from concourse.bass_utils import run_bass_kernel_spmd

    nc = build_nc()
    in_maps = make_in_maps(x, Wq, bq, Wk, bk, Wv, bv, Wo, bo)
    res = run_bass_kernel_spmd(nc, in_maps, core_ids=list(range(8)))
    bvo = Wo.astype(np.float32) @ bv.astype(np.float32) + bo.astype(np.float32)
    return assemble_out(res.results, bvo)
